# revision 35
# baseline (speedup 1.0000x reference)
"""CombinedRotaryEmbedding Trainium2 kernel.

Math (per 64-dim head, per position s, with R = composed Givens @ rotation_matrix):
    u = x @ R[:, 0::2],  v = x @ R[:, 1::2]
    out = [u*cos - v*sin | u*sin + v*cos]     cos/sin = f(position, freq[32])

Restructured as:  out = P .* CC + Q .* SS   with
    P = x @ [R_even | R_odd]        ([u|v])
    Q = x @ [-R_odd | R_even]       ([-v|u])
    CC = [cos|cos],  SS = [sin|sin]   (element-aligned halves, no swap needed)
so one fused matmul per 2-head chunk produces [P|Q] (free dim 256 -> float32r
runs at 1 cycle/row), DVE does one multiply pass, gpsimd one add pass.

Kernel strategy (8-way data parallel over the sequence dim):
  - host: tiny tables only: RR = [blockdiag(R2s,R2s) | blockdiag(R2sw,R2sw)]
    [128,256], compact ccss [128, 4, 2, 32] (cos/sin rows per position),
    and a 128x128 identity for the PE transposes.
  - device, per core (x shard [2048 rows, 1024] = 16 subtiles of 128 rows):
      SP   : all 16 subtile loads issued upfront (512 KB each), then stores;
             loads first means a store's sem wait can never delay a load
      PE   : transpose x chunks via f32r identity (1.5 cyc/row), then one
             f32r matmul per 2-head chunk -> [P|Q] (256 free, 1 cyc/row)
      ACT  : copy transposed chunks PSUM -> SBUF (4-chunk groups)
      DVE  : t = [P|Q] * [CC|SS]  (PSUM -> SBUF, [3,3,2]-chunk groups,
             sized by the 6 PSUM banks left next to the transpose tiles)
      GPSIMD: out = t_lo + t_hi per half-subtile, stored immediately
    The DVE multiply stream is the pacer (~41 us busy, gapless); the final
    subtile uses quarter-granularity adds/stores (last one on DVE) to
    shorten the tail after the last multiply.
"""

import numpy as np

import concourse.bacc as bacc
import concourse.bass as bass
import concourse.tile as tile
from concourse import mybir
from concourse.bass_utils import run_bass_kernel_spmd

N_CORES = 8
B, S, N_STATE = 4, 4096, 1024
H, D = 16, 64            # heads, head dim
HALF = D // 2            # 32 rotary freqs
S_SH = S // N_CORES      # 512 positions per core
ROWS = B * S_SH          # 2048 rows of [1024] per core
NBLK = ROWS // 256       # 8 DMA blocks of 256 rows
CBLK = S_SH // 128       # 4 distinct position blocks per core
NCH = N_STATE // 128     # 8 two-head chunks per row
F32 = mybir.dt.float32
F32R = mybir.dt.float32r
BF16 = mybir.dt.bfloat16

_compiled = {}


def _build_nc():
    nc = bacc.Bacc("TRN2")
    x_in = nc.dram_tensor("x", [ROWS, N_STATE], F32R, kind="ExternalInput")
    ident_in = nc.dram_tensor("ident", [128, 128], F32R, kind="ExternalInput")
    # RR = [blockdiag(R2s,R2s) | blockdiag(R2sw,R2sw)]: one K=128 matmul
    # yields [P|Q] for 2 heads
    rr_in = nc.dram_tensor("rr", [128, 256], F32R, kind="ExternalInput")
    # compact per-position rows: ccss[p, c, 0] = cos32, ccss[p, c, 1] = sin32
    ccss_in = nc.dram_tensor("ccss", [128, CBLK, 2, HALF], F32,
                             kind="ExternalInput")
    out_d = nc.dram_tensor("out", [ROWS, N_STATE], F32, kind="ExternalOutput")

    with tile.TileContext(nc) as tc:
        with (
            tc.tile_pool(name="const", bufs=1) as const,
            tc.tile_pool(name="xin", bufs=2 * NBLK) as xin,
            tc.tile_pool(name="xtp", bufs=3) as xtp,
            tc.tile_pool(name="tpsum", bufs=2, space="PSUM") as tpsum,
            tc.tile_pool(name="ypsum", bufs=2, space="PSUM") as ypsum,
            tc.tile_pool(name="tsb", bufs=3) as tsb,
            # one buffer per subtile: output buffers must never gate compute
            # (stores drain late because loads occupy the DMA engines first)
            tc.tile_pool(name="outp", bufs=2 * NBLK) as outp,
        ):
            # first subtile load goes ahead of everything else, split in two
            # so the transpose->copy->matmul->mul chain starts as early as
            # possible (chunks 0-3 usable after a 256 KB transfer)
            xts = []
            x_t = xin.tile([128, N_STATE], F32R)
            nc.sync.dma_start(out=x_t[:, 0:512], in_=x_in[0:128, 0:512])
            xts.append(x_t)

            ident = const.tile([128, 128], F32R)
            nc.sync.dma_start(out=ident[:], in_=ident_in[:])
            ccss_c = const.tile([128, CBLK, 2, HALF], F32)
            nc.sync.dma_start(out=ccss_c[:], in_=ccss_in[:])
            rr_sb = const.tile([128, 256], F32R)
            nc.sync.dma_start(out=rr_sb[:], in_=rr_in[:])
            nc.sync.dma_start(out=x_t[:, 512:1024], in_=x_in[0:128, 512:1024])

            # expand to [128, c, 2, 128]: row t repeated 4x along the free
            # dim; c=0 on DVE (gates the very first mul), rest on gpsimd to
            # keep them off the DVE's critical stream
            ccss_sb = const.tile([128, CBLK, 2, 128], F32)
            cbase = ccss_sb[:]
            sbase = ccss_c[:]
            for c in range(CBLK):
                eng = nc.vector if c == 0 else nc.gpsimd
                eng.tensor_copy(
                    bass.AP(tensor=cbase.tensor, offset=cbase.offset + c * 256,
                            ap=[list(cbase.ap[0]), [128, 2], [HALF, 4],
                                [1, HALF]]),
                    bass.AP(tensor=sbase.tensor, offset=sbase.offset + c * 64,
                            ap=[list(sbase.ap[0]), [HALF, 2], [0, 4],
                                [1, HALF]]),
                )

            # remaining subtile loads, all upfront: store waits can never
            # delay a load (single SP queue, loads issued first)
            for st in range(1, 2 * NBLK):
                x_t = xin.tile([128, N_STATE], F32R)
                nc.sync.dma_start(out=x_t[:],
                                  in_=x_in[st * 128:(st + 1) * 128, :])
                xts.append(x_t)

            for blk in range(NBLK):
                for j in range(2):
                    x_t = xts[2 * blk + j]
                    c = (2 * blk + j) % CBLK
                    xT = xtp.tile([128, NCH, 128], F32R)
                    t_sb = tsb.tile([128, NCH, 256], F32)
                    tb = t_sb[:]
                    o_t = outp.tile([128, N_STATE], F32)
                    ob = o_t[:]
                    def transpose_group(g):
                        # transpose 4 chunks: feats -> partitions
                        tp = tpsum.tile([128, 4, 128], F32R, tag="tp")
                        for q in range(4):
                            ch = 4 * g + q
                            nc.tensor.transpose(
                                tp[:, q, :],
                                x_t[:, ch * 128:(ch + 1) * 128],
                                ident[:],
                            )
                        nc.scalar.copy(out=xT[:, 4 * g:4 * (g + 1), :],
                                       in_=tp[:])

                    def transpose_chunks(g0, gn):
                        # per-mul-group staging: shortest startup chain
                        tp = tpsum.tile([128, 4, 128], F32R, tag="tp")
                        for a in range(gn):
                            ch = g0 + a
                            nc.tensor.transpose(
                                tp[:, a, :],
                                x_t[:, ch * 128:(ch + 1) * 128],
                                ident[:],
                            )
                        nc.scalar.copy(out=xT[:, g0:g0 + gn, :],
                                       in_=tp[:, 0:gn, :])

                    first = (blk == 0 and j == 0)
                    if not first:
                        transpose_group(0)
                    adds_done = 0
                    for g0, gn in ((0, 3), (3, 3), (6, 2)):
                        if first:
                            transpose_chunks(g0, gn)
                        elif g0 == 3:
                            # second transpose group after the first matmul
                            # group: PE unblocks DVE's group-0 mul earliest
                            transpose_group(1)
                        # [P|Q] per chunk; one matmul per PSUM bank
                        pq = ypsum.tile([128, 3, 512], F32)
                        for a in range(gn):
                            nc.tensor.matmul(
                                pq[:, a, 0:256], xT[:, g0 + a, :], rr_sb[:],
                                start=True, stop=True,
                            )
                        pqb = pq[:]
                        nc.vector.tensor_mul(
                            bass.AP(tensor=tb.tensor,
                                    offset=tb.offset + g0 * 256,
                                    ap=[list(tb.ap[0]), [256, gn], [1, 256]]),
                            bass.AP(tensor=pqb.tensor, offset=pqb.offset,
                                    ap=[list(pqb.ap[0]), [512, gn], [1, 256]]),
                            bass.AP(tensor=cbase.tensor,
                                    offset=cbase.offset + c * 256,
                                    ap=[list(cbase.ap[0]), [0, gn], [128, 2],
                                        [1, 128]]),
                        )
                        # ready chunks -> add + store; half-subtile granules
                        # normally, quarters on the final subtile so the tail
                        # after the very last mul is as short as possible
                        final = (blk == NBLK - 1 and j == 1)
                        gran = 2 if final else 4
                        while (g0 + gn) >= gran * (adds_done + 1):
                            h = adds_done
                            # trailing adds skip Pool's serial queue via DVE
                            eng = nc.vector if (final and h == 3) else nc.gpsimd
                            w = gran * 128
                            eng.tensor_tensor(
                                out=bass.AP(tensor=ob.tensor,
                                            offset=ob.offset + h * w,
                                            ap=[list(ob.ap[0]), [128, gran],
                                                [1, 128]]),
                                in0=bass.AP(tensor=tb.tensor,
                                            offset=tb.offset + h * gran * 256,
                                            ap=[list(tb.ap[0]), [256, gran],
                                                [1, 128]]),
                                in1=bass.AP(tensor=tb.tensor,
                                            offset=tb.offset + h * gran * 256
                                            + 128,
                                            ap=[list(tb.ap[0]), [256, gran],
                                                [1, 128]]),
                                op=mybir.AluOpType.add,
                            )
                            r0 = blk * 256 + j * 128
                            nc.sync.dma_start(
                                out=out_d[r0:r0 + 128, h * w:(h + 1) * w],
                                in_=o_t[:, h * w:(h + 1) * w])
                            adds_done += 1
    nc.compile()  # bacc: splits multi-sem waits into EventSemaphore insts
    return nc


def _compose_r(thetas, rotation_pairs, theta_scale, rotation_matrix):
    """Replicates reference._compose_rotation."""
    idx = rotation_pairs.astype(np.int32)
    th = thetas.astype(np.float32) * np.float32(theta_scale[0])
    R = np.eye(D, dtype=np.float32)
    for k in range(th.shape[0]):
        i, j = int(idx[k, 0]), int(idx[k, 1])
        ck, sk = np.float32(np.cos(th[k])), np.float32(np.sin(th[k]))
        G = np.eye(D, dtype=np.float32)
        G[i, i] = ck
        G[i, j] = -sk
        G[j, i] = sk
        G[j, j] = ck
        R = (R @ G).astype(np.float32)
    return (R @ rotation_matrix.astype(np.float32)).astype(np.float32)


def _build_rr(R):
    """[128, 256] = [blockdiag(R2s,R2s) | blockdiag(R2sw,R2sw)] where
    R2s = [R_even|R_odd] (-> P = [u|v]) and R2sw = [-R_odd|R_even]
    (-> Q = [-v|u])."""
    r2s = np.concatenate([R[:, 0::2], R[:, 1::2]], axis=1)
    r2sw = np.concatenate([-R[:, 1::2], R[:, 0::2]], axis=1)

    def blkdiag(m):
        z = np.zeros((128, 128), dtype=np.float32)
        z[0:D, 0:D] = m
        z[D:128, D:128] = m
        return z

    return np.ascontiguousarray(
        np.concatenate([blkdiag(r2s), blkdiag(r2sw)], axis=1), dtype=np.float32)


def make_in_maps(x, thetas, rotation_pairs, theta_scale, rotation_matrix,
                 inv_freq):
    x = np.asarray(x, dtype=np.float32)
    R = _compose_r(
        np.asarray(thetas, np.float32),
        np.asarray(rotation_pairs, np.float32),
        np.asarray(theta_scale, np.float32),
        np.asarray(rotation_matrix, np.float32),
    )
    rr = _build_rr(R)
    invf = np.asarray(inv_freq, np.float32)
    pos = np.arange(S, dtype=np.float32)
    sinusoid = pos[:, None] * invf[None, :]               # [S, 32]
    cosf = np.cos(sinusoid).astype(np.float32)
    sinf = np.sin(sinusoid).astype(np.float32)

    in_maps = []
    for k in range(N_CORES):
        blk = slice(k * S_SH, (k + 1) * S_SH)
        # ccss[p, c, 0] = cos row, ccss[p, c, 1] = sin row (pos = c*128 + p)
        cc = cosf[blk].reshape(CBLK, 128, HALF)
        ss = sinf[blk].reshape(CBLK, 128, HALF)
        ccss = np.ascontiguousarray(
            np.stack([cc, ss], axis=2).transpose(1, 0, 2, 3), dtype=np.float32)
        xs = np.ascontiguousarray(x[:, blk, :]).reshape(ROWS, N_STATE)
        in_maps.append({"x": xs, "rr": rr, "ccss": ccss,
                        "ident": np.eye(128, dtype=np.float32)})
    return in_maps


def kernel(x, thetas, rotation_pairs, theta_scale, rotation_matrix, inv_freq):
    in_maps = make_in_maps(x, thetas, rotation_pairs, theta_scale,
                           rotation_matrix, inv_freq)
    if "nc" not in _compiled:
        _compiled["nc"] = _build_nc()
    res = run_bass_kernel_spmd(_compiled["nc"], in_maps,
                               list(range(N_CORES))).results

    out = np.empty((B, S, N_STATE), dtype=np.float32)
    for k in range(N_CORES):
        blk = slice(k * S_SH, (k + 1) * S_SH)
        out[:, blk, :] = res[k]["out"].reshape(B, S_SH, N_STATE)
    return out


# revision 41
# speedup vs baseline: 1.0162x; 1.0162x over previous
"""CombinedRotaryEmbedding Trainium2 kernel.

Math (per 64-dim head, per position s, with R = composed Givens @ rotation_matrix):
    u = x @ R[:, 0::2],  v = x @ R[:, 1::2]
    out = [u*cos - v*sin | u*sin + v*cos]     cos/sin = f(position, freq[32])

Restructured as:  out = P .* CC + Q .* SS   with
    P = x @ [R_even | R_odd]        ([u|v])
    Q = x @ [-R_odd | R_even]       ([-v|u])
    CC = [cos|cos],  SS = [sin|sin]   (element-aligned halves, no swap needed)
so one fused matmul per 2-head chunk produces [P|Q] (free dim 256 -> float32r
runs at 1 cycle/row), DVE does one multiply pass, gpsimd one add pass.

Kernel strategy (8-way data parallel over the sequence dim):
  - host: tiny tables only: RR = [blockdiag(R2s,R2s) | blockdiag(R2sw,R2sw)]
    [128,256], compact ccss [128, 4, 2, 32] (cos/sin rows per position),
    and a 128x128 identity for the PE transposes.
  - device, per core (x shard [2048 rows, 1024] = 16 subtiles of 128 rows):
      SP   : all 16 subtile loads issued upfront (512 KB each), then stores;
             loads first means a store's sem wait can never delay a load
      PE   : transpose x chunks via f32r identity (1.5 cyc/row), then one
             f32r matmul per 2-head chunk -> [P|Q] (256 free, 1 cyc/row)
      ACT  : copy transposed chunks PSUM -> SBUF (4-chunk groups)
      DVE  : t = [P|Q] * [CC|SS]  (PSUM -> SBUF, [3,3,2]-chunk groups,
             sized by the 6 PSUM banks left next to the transpose tiles)
      GPSIMD: out = t_lo + t_hi per half-subtile, stored immediately
    The DVE multiply stream is the pacer (~41 us busy, gapless); the final
    subtile uses quarter-granularity adds/stores (last one on DVE) to
    shorten the tail after the last multiply.
"""

import numpy as np

import concourse.bacc as bacc
import concourse.bass as bass
import concourse.tile as tile
from concourse import mybir
from concourse.bass_utils import run_bass_kernel_spmd

N_CORES = 8
B, S, N_STATE = 4, 4096, 1024
H, D = 16, 64            # heads, head dim
HALF = D // 2            # 32 rotary freqs
S_SH = S // N_CORES      # 512 positions per core
ROWS = B * S_SH          # 2048 rows of [1024] per core
NBLK = ROWS // 256       # 8 DMA blocks of 256 rows
CBLK = S_SH // 128       # 4 distinct position blocks per core
NCH = N_STATE // 128     # 8 two-head chunks per row
F32 = mybir.dt.float32
F32R = mybir.dt.float32r
BF16 = mybir.dt.bfloat16

_compiled = {}


def _build_nc():
    nc = bacc.Bacc("TRN2")
    x_in = nc.dram_tensor("x", [ROWS, N_STATE], F32R, kind="ExternalInput")
    ident_in = nc.dram_tensor("ident", [128, 128], F32R, kind="ExternalInput")
    # RR = [blockdiag(R2s,R2s) | blockdiag(R2sw,R2sw)]: one K=128 matmul
    # yields [P|Q] for 2 heads
    rr_in = nc.dram_tensor("rr", [128, 256], F32R, kind="ExternalInput")
    # compact per-position rows: ccss[p, c, 0] = cos32, ccss[p, c, 1] = sin32
    ccss_in = nc.dram_tensor("ccss", [128, CBLK, 2, HALF], F32,
                             kind="ExternalInput")
    out_d = nc.dram_tensor("out", [ROWS, N_STATE], F32, kind="ExternalOutput")

    with tile.TileContext(nc) as tc:
        with (
            tc.tile_pool(name="const", bufs=1) as const,
            tc.tile_pool(name="xin", bufs=2 * NBLK) as xin,
            tc.tile_pool(name="xtp", bufs=3) as xtp,
            tc.tile_pool(name="tpsum", bufs=2, space="PSUM") as tpsum,
            tc.tile_pool(name="ypsum", bufs=2, space="PSUM") as ypsum,
            tc.tile_pool(name="tsb", bufs=3) as tsb,
            # one buffer per subtile: output buffers must never gate compute
            # (stores drain late because loads occupy the DMA engines first)
            tc.tile_pool(name="outp", bufs=2 * NBLK) as outp,
        ):
            # first subtile load goes ahead of everything else, split in two
            # so the transpose->copy->matmul->mul chain starts as early as
            # possible (chunks 0-3 usable after a 256 KB transfer)
            xts = []
            x_t = xin.tile([128, N_STATE], F32R)
            nc.sync.dma_start(out=x_t[:, 0:512], in_=x_in[0:128, 0:512])
            xts.append(x_t)

            ident = const.tile([128, 128], F32R)
            nc.sync.dma_start(out=ident[:], in_=ident_in[:])
            ccss_c = const.tile([128, CBLK, 2, HALF], F32)
            nc.sync.dma_start(out=ccss_c[:], in_=ccss_in[:])
            rr_sb = const.tile([128, 256], F32R)
            nc.sync.dma_start(out=rr_sb[:], in_=rr_in[:])
            nc.sync.dma_start(out=x_t[:, 512:1024], in_=x_in[0:128, 512:1024])

            # expand to [128, c, 2, 128]: row t repeated 4x along the free
            # dim; c=0 on DVE (gates the very first mul), rest on gpsimd to
            # keep them off the DVE's critical stream
            ccss_sb = const.tile([128, CBLK, 2, 128], F32)
            cbase = ccss_sb[:]
            sbase = ccss_c[:]
            for c in range(CBLK):
                eng = nc.vector if c == 0 else nc.gpsimd
                eng.tensor_copy(
                    bass.AP(tensor=cbase.tensor, offset=cbase.offset + c * 256,
                            ap=[list(cbase.ap[0]), [128, 2], [HALF, 4],
                                [1, HALF]]),
                    bass.AP(tensor=sbase.tensor, offset=sbase.offset + c * 64,
                            ap=[list(sbase.ap[0]), [HALF, 2], [0, 4],
                                [1, HALF]]),
                )

            # remaining subtile loads, all upfront: store waits can never
            # delay a load (single SP queue, loads issued first); subtiles
            # 1-2 split in half so the still-filling pipeline starves less
            for st in range(1, 2 * NBLK):
                x_t = xin.tile([128, N_STATE], F32R)
                if st <= 2:
                    nc.sync.dma_start(out=x_t[:, 0:512],
                                      in_=x_in[st * 128:(st + 1) * 128, 0:512])
                    nc.sync.dma_start(out=x_t[:, 512:1024],
                                      in_=x_in[st * 128:(st + 1) * 128,
                                               512:1024])
                else:
                    nc.sync.dma_start(out=x_t[:],
                                      in_=x_in[st * 128:(st + 1) * 128, :])
                xts.append(x_t)

            for blk in range(NBLK):
                for j in range(2):
                    x_t = xts[2 * blk + j]
                    c = (2 * blk + j) % CBLK
                    xT = xtp.tile([128, NCH, 128], F32R)
                    t_sb = tsb.tile([128, NCH, 256], F32)
                    tb = t_sb[:]
                    o_t = outp.tile([128, N_STATE], F32)
                    ob = o_t[:]
                    def transpose_group(g):
                        # transpose 4 chunks: feats -> partitions
                        tp = tpsum.tile([128, 4, 128], F32R, tag="tp")
                        for q in range(4):
                            ch = 4 * g + q
                            nc.tensor.transpose(
                                tp[:, q, :],
                                x_t[:, ch * 128:(ch + 1) * 128],
                                ident[:],
                            )
                        nc.scalar.copy(out=xT[:, 4 * g:4 * (g + 1), :],
                                       in_=tp[:])

                    def transpose_chunks(g0, gn):
                        # per-mul-group staging: shortest startup chain
                        tp = tpsum.tile([128, 4, 128], F32R, tag="tp")
                        for a in range(gn):
                            ch = g0 + a
                            nc.tensor.transpose(
                                tp[:, a, :],
                                x_t[:, ch * 128:(ch + 1) * 128],
                                ident[:],
                            )
                        nc.scalar.copy(out=xT[:, g0:g0 + gn, :],
                                       in_=tp[:, 0:gn, :])

                    first = (blk == 0 and j == 0)
                    if not first:
                        transpose_group(0)
                    adds_done = 0
                    groups = ((0, 2), (2, 2), (4, 2), (6, 2)) if first else \
                        ((0, 3), (3, 3), (6, 2))
                    for g0, gn in groups:
                        if first:
                            transpose_chunks(g0, gn)
                        elif g0 == 3:
                            # second transpose group after the first matmul
                            # group: PE unblocks DVE's group-0 mul earliest
                            transpose_group(1)
                        # [P|Q] per chunk; one matmul per PSUM bank
                        pq = ypsum.tile([128, 3, 512], F32)
                        for a in range(gn):
                            nc.tensor.matmul(
                                pq[:, a, 0:256], xT[:, g0 + a, :], rr_sb[:],
                                start=True, stop=True,
                            )
                        pqb = pq[:]
                        nc.vector.tensor_mul(
                            bass.AP(tensor=tb.tensor,
                                    offset=tb.offset + g0 * 256,
                                    ap=[list(tb.ap[0]), [256, gn], [1, 256]]),
                            bass.AP(tensor=pqb.tensor, offset=pqb.offset,
                                    ap=[list(pqb.ap[0]), [512, gn], [1, 256]]),
                            bass.AP(tensor=cbase.tensor,
                                    offset=cbase.offset + c * 256,
                                    ap=[list(cbase.ap[0]), [0, gn], [128, 2],
                                        [1, 128]]),
                        )
                        # ready chunks -> add + store; half-subtile granules
                        # normally, quarters on the final subtile so the tail
                        # after the very last mul is as short as possible
                        final = (blk == NBLK - 1 and j == 1)
                        gran = 2 if final else 4
                        while (g0 + gn) >= gran * (adds_done + 1):
                            h = adds_done
                            # trailing adds skip Pool's serial queue via DVE
                            eng = nc.vector if (final and h == 3) else nc.gpsimd
                            w = gran * 128
                            eng.tensor_tensor(
                                out=bass.AP(tensor=ob.tensor,
                                            offset=ob.offset + h * w,
                                            ap=[list(ob.ap[0]), [128, gran],
                                                [1, 128]]),
                                in0=bass.AP(tensor=tb.tensor,
                                            offset=tb.offset + h * gran * 256,
                                            ap=[list(tb.ap[0]), [256, gran],
                                                [1, 128]]),
                                in1=bass.AP(tensor=tb.tensor,
                                            offset=tb.offset + h * gran * 256
                                            + 128,
                                            ap=[list(tb.ap[0]), [256, gran],
                                                [1, 128]]),
                                op=mybir.AluOpType.add,
                            )
                            r0 = blk * 256 + j * 128
                            nc.sync.dma_start(
                                out=out_d[r0:r0 + 128, h * w:(h + 1) * w],
                                in_=o_t[:, h * w:(h + 1) * w])
                            adds_done += 1
    nc.compile()  # bacc: splits multi-sem waits into EventSemaphore insts
    return nc


def _compose_r(thetas, rotation_pairs, theta_scale, rotation_matrix):
    """Replicates reference._compose_rotation."""
    idx = rotation_pairs.astype(np.int32)
    th = thetas.astype(np.float32) * np.float32(theta_scale[0])
    R = np.eye(D, dtype=np.float32)
    for k in range(th.shape[0]):
        i, j = int(idx[k, 0]), int(idx[k, 1])
        ck, sk = np.float32(np.cos(th[k])), np.float32(np.sin(th[k]))
        G = np.eye(D, dtype=np.float32)
        G[i, i] = ck
        G[i, j] = -sk
        G[j, i] = sk
        G[j, j] = ck
        R = (R @ G).astype(np.float32)
    return (R @ rotation_matrix.astype(np.float32)).astype(np.float32)


def _build_rr(R):
    """[128, 256] = [blockdiag(R2s,R2s) | blockdiag(R2sw,R2sw)] where
    R2s = [R_even|R_odd] (-> P = [u|v]) and R2sw = [-R_odd|R_even]
    (-> Q = [-v|u])."""
    r2s = np.concatenate([R[:, 0::2], R[:, 1::2]], axis=1)
    r2sw = np.concatenate([-R[:, 1::2], R[:, 0::2]], axis=1)

    def blkdiag(m):
        z = np.zeros((128, 128), dtype=np.float32)
        z[0:D, 0:D] = m
        z[D:128, D:128] = m
        return z

    return np.ascontiguousarray(
        np.concatenate([blkdiag(r2s), blkdiag(r2sw)], axis=1), dtype=np.float32)


def make_in_maps(x, thetas, rotation_pairs, theta_scale, rotation_matrix,
                 inv_freq):
    x = np.asarray(x, dtype=np.float32)
    R = _compose_r(
        np.asarray(thetas, np.float32),
        np.asarray(rotation_pairs, np.float32),
        np.asarray(theta_scale, np.float32),
        np.asarray(rotation_matrix, np.float32),
    )
    rr = _build_rr(R)
    invf = np.asarray(inv_freq, np.float32)
    pos = np.arange(S, dtype=np.float32)
    sinusoid = pos[:, None] * invf[None, :]               # [S, 32]
    cosf = np.cos(sinusoid).astype(np.float32)
    sinf = np.sin(sinusoid).astype(np.float32)

    in_maps = []
    for k in range(N_CORES):
        blk = slice(k * S_SH, (k + 1) * S_SH)
        # ccss[p, c, 0] = cos row, ccss[p, c, 1] = sin row (pos = c*128 + p)
        cc = cosf[blk].reshape(CBLK, 128, HALF)
        ss = sinf[blk].reshape(CBLK, 128, HALF)
        ccss = np.ascontiguousarray(
            np.stack([cc, ss], axis=2).transpose(1, 0, 2, 3), dtype=np.float32)
        xs = np.ascontiguousarray(x[:, blk, :]).reshape(ROWS, N_STATE)
        in_maps.append({"x": xs, "rr": rr, "ccss": ccss,
                        "ident": np.eye(128, dtype=np.float32)})
    return in_maps


def kernel(x, thetas, rotation_pairs, theta_scale, rotation_matrix, inv_freq):
    in_maps = make_in_maps(x, thetas, rotation_pairs, theta_scale,
                           rotation_matrix, inv_freq)
    if "nc" not in _compiled:
        _compiled["nc"] = _build_nc()
    res = run_bass_kernel_spmd(_compiled["nc"], in_maps,
                               list(range(N_CORES))).results

    out = np.empty((B, S, N_STATE), dtype=np.float32)
    for k in range(N_CORES):
        blk = slice(k * S_SH, (k + 1) * S_SH)
        out[:, blk, :] = res[k]["out"].reshape(B, S_SH, N_STATE)
    return out


# revision 56
# speedup vs baseline: 1.0191x; 1.0029x over previous
"""CombinedRotaryEmbedding Trainium2 kernel.

Math (per 64-dim head, per position s, with R = composed Givens @ rotation_matrix):
    u = x @ R[:, 0::2],  v = x @ R[:, 1::2]
    out = [u*cos - v*sin | u*sin + v*cos]     cos/sin = f(position, freq[32])

Restructured as:  out = P .* CC + Q .* SS   with
    P = x @ [R_even | R_odd]        ([u|v])
    Q = x @ [-R_odd | R_even]       ([-v|u])
    CC = [cos|cos],  SS = [sin|sin]   (element-aligned halves, no swap needed)
so one fused matmul per 2-head chunk produces [P|Q] (free dim 256 -> float32r
runs at 1 cycle/row), DVE does one multiply pass, gpsimd one add pass.

Kernel strategy (8-way data parallel over the sequence dim):
  - host: tiny tables only: RR = [blockdiag(R2s,R2s) | blockdiag(R2sw,R2sw)]
    [128,256], compact ccss [128, 4, 2, 32] (cos/sin rows per position),
    and a 128x128 identity for the PE transposes.
  - device, per core (x shard [2048 rows, 1024] = 16 subtiles of 128 rows):
      SP   : all 16 subtile loads issued upfront (512 KB each), then stores;
             loads first means a store's sem wait can never delay a load
      PE   : transpose x chunks via f32r identity (1.5 cyc/row), then one
             f32r matmul per 2-head chunk -> [P|Q] (256 free, 1 cyc/row)
      ACT  : copy transposed chunks PSUM -> SBUF (4-chunk groups)
      DVE  : t = [P|Q] * [CC|SS]  (PSUM -> SBUF, [3,3,2]-chunk groups,
             sized by the 6 PSUM banks left next to the transpose tiles)
      GPSIMD: out = t_lo + t_hi per half-subtile, stored immediately
    The DVE multiply stream is the pacer (~41 us busy, gapless); the final
    subtile uses quarter-granularity adds/stores (last one on DVE) to
    shorten the tail after the last multiply.
"""

import numpy as np

import concourse.bacc as bacc
import concourse.bass as bass
import concourse.tile as tile
from concourse import mybir
from concourse.bass_utils import run_bass_kernel_spmd

N_CORES = 8
B, S, N_STATE = 4, 4096, 1024
H, D = 16, 64            # heads, head dim
HALF = D // 2            # 32 rotary freqs
S_SH = S // N_CORES      # 512 positions per core
ROWS = B * S_SH          # 2048 rows of [1024] per core
NBLK = ROWS // 256       # 8 DMA blocks of 256 rows
CBLK = S_SH // 128       # 4 distinct position blocks per core
NCH = N_STATE // 128     # 8 two-head chunks per row
F32 = mybir.dt.float32
F32R = mybir.dt.float32r
BF16 = mybir.dt.bfloat16

_compiled = {}


def _build_nc():
    nc = bacc.Bacc("TRN2")
    x_in = nc.dram_tensor("x", [ROWS, N_STATE], F32R, kind="ExternalInput")
    ident_in = nc.dram_tensor("ident", [128, 128], F32R, kind="ExternalInput")
    # RR = [blockdiag(R2s,R2s) | blockdiag(R2sw,R2sw)]: one K=128 matmul
    # yields [P|Q] for 2 heads
    rr_in = nc.dram_tensor("rr", [128, 256], F32R, kind="ExternalInput")
    # compact per-position rows: ccss[p, c, 0] = cos32, ccss[p, c, 1] = sin32
    ccss_in = nc.dram_tensor("ccss", [128, CBLK, 2, HALF], F32,
                             kind="ExternalInput")
    out_d = nc.dram_tensor("out", [ROWS, N_STATE], F32, kind="ExternalOutput")

    with tile.TileContext(nc) as tc:
        with (
            tc.tile_pool(name="const", bufs=1) as const,
            tc.tile_pool(name="xin", bufs=2 * NBLK) as xin,
            tc.tile_pool(name="xtp", bufs=3) as xtp,
            tc.tile_pool(name="tpsum", bufs=2, space="PSUM") as tpsum,
            tc.tile_pool(name="ypsum", bufs=2, space="PSUM") as ypsum,
            tc.tile_pool(name="tsb", bufs=3) as tsb,
            # one buffer per subtile: output buffers must never gate compute
            # (stores drain late because loads occupy the DMA engines first)
            tc.tile_pool(name="outp", bufs=2 * NBLK) as outp,
        ):
            # first subtile load goes ahead of everything else, split in two
            # so the transpose->copy->matmul->mul chain starts as early as
            # possible (chunks 0-3 usable after a 256 KB transfer)
            xts = []
            x_t = xin.tile([128, N_STATE], F32R, tag="xin", name="xs0")
            nc.sync.dma_start(out=x_t[:, 0:512], in_=x_in[0:128, 0:512])
            xts.append(x_t)

            ident = const.tile([128, 128], F32R)
            nc.sync.dma_start(out=ident[:], in_=ident_in[:])
            ccss_c = const.tile([128, CBLK, 2, HALF], F32)
            nc.sync.dma_start(out=ccss_c[:], in_=ccss_in[:])
            rr_sb = const.tile([128, 256], F32R)
            nc.sync.dma_start(out=rr_sb[:], in_=rr_in[:])
            nc.sync.dma_start(out=x_t[:, 512:1024], in_=x_in[0:128, 512:1024])

            # expand to [128, c, 2, 128]: row t repeated 4x along the free
            # dim; c=0 on DVE (gates the very first mul), rest on gpsimd to
            # keep them off the DVE's critical stream
            ccss_sb = const.tile([128, CBLK, 2, 128], F32)
            cbase = ccss_sb[:]
            sbase = ccss_c[:]
            for c in range(CBLK):
                eng = nc.vector if c == 0 else nc.gpsimd
                eng.tensor_copy(
                    bass.AP(tensor=cbase.tensor, offset=cbase.offset + c * 256,
                            ap=[list(cbase.ap[0]), [128, 2], [HALF, 4],
                                [1, HALF]]),
                    bass.AP(tensor=sbase.tensor, offset=sbase.offset + c * 64,
                            ap=[list(sbase.ap[0]), [HALF, 2], [0, 4],
                                [1, HALF]]),
                )

            # remaining subtile loads, all upfront: store waits can never
            # delay a load (single SP queue, loads issued first); subtiles
            # 1-2 split in half so the still-filling pipeline starves less
            for st in range(1, 2 * NBLK):
                xts.append(xin.tile([128, N_STATE], F32R, tag="xin",
                                    name=f"xs{st}"))
            # subtiles 1-2 split in half so the filling pipeline
            # starves less
            for st in (1, 2):
                nc.sync.dma_start(out=xts[st][:, 0:512],
                                  in_=x_in[st * 128:(st + 1) * 128, 0:512])
                nc.sync.dma_start(out=xts[st][:, 512:1024],
                                  in_=x_in[st * 128:(st + 1) * 128, 512:1024])
            for st in range(3, 2 * NBLK):
                nc.sync.dma_start(out=xts[st][:],
                                  in_=x_in[st * 128:(st + 1) * 128, :])

            for blk in range(NBLK):
                for j in range(2):
                    x_t = xts[2 * blk + j]
                    c = (2 * blk + j) % CBLK
                    xT = xtp.tile([128, NCH, 128], F32R)
                    t_sb = tsb.tile([128, NCH, 256], F32)
                    tb = t_sb[:]
                    o_t = outp.tile([128, N_STATE], F32)
                    ob = o_t[:]
                    def transpose_group(g):
                        # transpose 4 chunks: feats -> partitions
                        tp = tpsum.tile([128, 4, 128], F32R, tag="tp")
                        for q in range(4):
                            ch = 4 * g + q
                            nc.tensor.transpose(
                                tp[:, q, :],
                                x_t[:, ch * 128:(ch + 1) * 128],
                                ident[:],
                            )
                        nc.scalar.copy(out=xT[:, 4 * g:4 * (g + 1), :],
                                       in_=tp[:])

                    def transpose_chunks(g0, gn):
                        # per-mul-group staging: shortest startup chain
                        tp = tpsum.tile([128, 4, 128], F32R, tag="tp")
                        for a in range(gn):
                            ch = g0 + a
                            nc.tensor.transpose(
                                tp[:, a, :],
                                x_t[:, ch * 128:(ch + 1) * 128],
                                ident[:],
                            )
                        nc.scalar.copy(out=xT[:, g0:g0 + gn, :],
                                       in_=tp[:, 0:gn, :])

                    first = (blk == 0 and j == 0)
                    if not first:
                        transpose_group(0)
                    adds_done = 0
                    groups = ((0, 2), (2, 2), (4, 2), (6, 2)) if first else \
                        ((0, 3), (3, 3), (6, 2))
                    for g0, gn in groups:
                        if first:
                            transpose_chunks(g0, gn)
                        elif g0 == 3:
                            # second transpose group after the first matmul
                            # group: PE unblocks DVE's group-0 mul earliest
                            transpose_group(1)
                        # [P|Q] per chunk; one matmul per PSUM bank
                        pq = ypsum.tile([128, 3, 512], F32)
                        for a in range(gn):
                            nc.tensor.matmul(
                                pq[:, a, 0:256], xT[:, g0 + a, :], rr_sb[:],
                                start=True, stop=True,
                            )
                        pqb = pq[:]
                        nc.vector.tensor_mul(
                            bass.AP(tensor=tb.tensor,
                                    offset=tb.offset + g0 * 256,
                                    ap=[list(tb.ap[0]), [256, gn], [1, 256]]),
                            bass.AP(tensor=pqb.tensor, offset=pqb.offset,
                                    ap=[list(pqb.ap[0]), [512, gn], [1, 256]]),
                            bass.AP(tensor=cbase.tensor,
                                    offset=cbase.offset + c * 256,
                                    ap=[list(cbase.ap[0]), [0, gn], [128, 2],
                                        [1, 128]]),
                        )
                        # ready chunks -> add + store; half-subtile granules
                        # normally, quarters on the final subtile so the tail
                        # after the very last mul is as short as possible
                        final = (blk == NBLK - 1 and j == 1)
                        gran = 4
                        while (g0 + gn) >= gran * (adds_done + 1):
                            h = adds_done
                            # trailing adds skip Pool's serial queue via DVE
                            eng = nc.vector if (final and h == 1) else nc.gpsimd
                            w = gran * 128
                            eng.tensor_tensor(
                                out=bass.AP(tensor=ob.tensor,
                                            offset=ob.offset + h * w,
                                            ap=[list(ob.ap[0]), [128, gran],
                                                [1, 128]]),
                                in0=bass.AP(tensor=tb.tensor,
                                            offset=tb.offset + h * gran * 256,
                                            ap=[list(tb.ap[0]), [256, gran],
                                                [1, 128]]),
                                in1=bass.AP(tensor=tb.tensor,
                                            offset=tb.offset + h * gran * 256
                                            + 128,
                                            ap=[list(tb.ap[0]), [256, gran],
                                                [1, 128]]),
                                op=mybir.AluOpType.add,
                            )
                            r0 = blk * 256 + j * 128
                            nc.sync.dma_start(
                                out=out_d[r0:r0 + 128, h * w:(h + 1) * w],
                                in_=o_t[:, h * w:(h + 1) * w])
                            adds_done += 1
    nc.compile()  # bacc: splits multi-sem waits into EventSemaphore insts
    return nc


def _compose_r(thetas, rotation_pairs, theta_scale, rotation_matrix):
    """Replicates reference._compose_rotation."""
    idx = rotation_pairs.astype(np.int32)
    th = thetas.astype(np.float32) * np.float32(theta_scale[0])
    R = np.eye(D, dtype=np.float32)
    for k in range(th.shape[0]):
        i, j = int(idx[k, 0]), int(idx[k, 1])
        ck, sk = np.float32(np.cos(th[k])), np.float32(np.sin(th[k]))
        G = np.eye(D, dtype=np.float32)
        G[i, i] = ck
        G[i, j] = -sk
        G[j, i] = sk
        G[j, j] = ck
        R = (R @ G).astype(np.float32)
    return (R @ rotation_matrix.astype(np.float32)).astype(np.float32)


def _build_rr(R):
    """[128, 256] = [blockdiag(R2s,R2s) | blockdiag(R2sw,R2sw)] where
    R2s = [R_even|R_odd] (-> P = [u|v]) and R2sw = [-R_odd|R_even]
    (-> Q = [-v|u])."""
    r2s = np.concatenate([R[:, 0::2], R[:, 1::2]], axis=1)
    r2sw = np.concatenate([-R[:, 1::2], R[:, 0::2]], axis=1)

    def blkdiag(m):
        z = np.zeros((128, 128), dtype=np.float32)
        z[0:D, 0:D] = m
        z[D:128, D:128] = m
        return z

    return np.ascontiguousarray(
        np.concatenate([blkdiag(r2s), blkdiag(r2sw)], axis=1), dtype=np.float32)


def make_in_maps(x, thetas, rotation_pairs, theta_scale, rotation_matrix,
                 inv_freq):
    x = np.asarray(x, dtype=np.float32)
    R = _compose_r(
        np.asarray(thetas, np.float32),
        np.asarray(rotation_pairs, np.float32),
        np.asarray(theta_scale, np.float32),
        np.asarray(rotation_matrix, np.float32),
    )
    rr = _build_rr(R)
    invf = np.asarray(inv_freq, np.float32)
    pos = np.arange(S, dtype=np.float32)
    sinusoid = pos[:, None] * invf[None, :]               # [S, 32]
    cosf = np.cos(sinusoid).astype(np.float32)
    sinf = np.sin(sinusoid).astype(np.float32)

    in_maps = []
    for k in range(N_CORES):
        blk = slice(k * S_SH, (k + 1) * S_SH)
        # ccss[p, c, 0] = cos row, ccss[p, c, 1] = sin row (pos = c*128 + p)
        cc = cosf[blk].reshape(CBLK, 128, HALF)
        ss = sinf[blk].reshape(CBLK, 128, HALF)
        ccss = np.ascontiguousarray(
            np.stack([cc, ss], axis=2).transpose(1, 0, 2, 3), dtype=np.float32)
        xs = np.ascontiguousarray(x[:, blk, :]).reshape(ROWS, N_STATE)
        in_maps.append({"x": xs, "rr": rr, "ccss": ccss,
                        "ident": np.eye(128, dtype=np.float32)})
    return in_maps


def kernel(x, thetas, rotation_pairs, theta_scale, rotation_matrix, inv_freq):
    in_maps = make_in_maps(x, thetas, rotation_pairs, theta_scale,
                           rotation_matrix, inv_freq)
    if "nc" not in _compiled:
        _compiled["nc"] = _build_nc()
    res = run_bass_kernel_spmd(_compiled["nc"], in_maps,
                               list(range(N_CORES))).results

    out = np.empty((B, S, N_STATE), dtype=np.float32)
    for k in range(N_CORES):
        blk = slice(k * S_SH, (k + 1) * S_SH)
        out[:, blk, :] = res[k]["out"].reshape(B, S_SH, N_STATE)
    return out


# revision 61
# speedup vs baseline: 1.0502x; 1.0304x over previous
"""CombinedRotaryEmbedding Trainium2 kernel.

Math (per 64-dim head, per position s, with R = composed Givens @ rotation_matrix):
    u = x @ R[:, 0::2],  v = x @ R[:, 1::2]
    out = [u*cos - v*sin | u*sin + v*cos]     cos/sin = f(position, freq[32])

Restructured as:  out = P .* CC + Q .* SS   with
    P = x @ [R_even | R_odd]        ([u|v])
    Q = x @ [-R_odd | R_even]       ([-v|u])
    CC = [cos|cos],  SS = [sin|sin]   (element-aligned halves, no swap needed)
so one fused matmul per 2-head chunk produces [P|Q] (free dim 256 -> float32r
runs at 1 cycle/row), DVE does one multiply pass, gpsimd one add pass.

Kernel strategy (8-way data parallel over the sequence dim):
  - host: tiny tables only: RR = [blockdiag(R2s,R2s) | blockdiag(R2sw,R2sw)]
    [128,256], compact ccss [128, 4, 2, 32] (cos/sin rows per position),
    and a 128x128 identity for the PE transposes.
  - device, per core (x shard [2048 rows, 1024] = 16 subtiles of 128 rows):
      SP   : all 16 subtile loads issued upfront (512 KB each), then stores;
             loads first means a store's sem wait can never delay a load
      PE   : transpose x chunks via f32r identity (1.5 cyc/row), then one
             f32r matmul per 2-head chunk -> [P|Q] (256 free, 1 cyc/row)
      ACT  : copy transposed chunks PSUM -> SBUF (4-chunk groups)
      DVE  : t = [P|Q] * [CC|SS]  (PSUM -> SBUF, [3,3,2]-chunk groups,
             sized by the 6 PSUM banks left next to the transpose tiles)
      GPSIMD: out = t_lo + t_hi per half-subtile, stored immediately
    The DVE multiply stream is the pacer (~41 us busy, gapless); the final
    subtile uses quarter-granularity adds/stores (last one on DVE) to
    shorten the tail after the last multiply.
"""

import numpy as np

import concourse.bacc as bacc
import concourse.bass as bass
import concourse.tile as tile
from concourse import mybir
from concourse.bass_utils import run_bass_kernel_spmd

N_CORES = 8
B, S, N_STATE = 4, 4096, 1024
H, D = 16, 64            # heads, head dim
HALF = D // 2            # 32 rotary freqs
S_SH = S // N_CORES      # 512 positions per core
ROWS = B * S_SH          # 2048 rows of [1024] per core
NBLK = ROWS // 256       # 8 DMA blocks of 256 rows
CBLK = S_SH // 128       # 4 distinct position blocks per core
NCH = N_STATE // 128     # 8 two-head chunks per row
F32 = mybir.dt.float32
F32R = mybir.dt.float32r
BF16 = mybir.dt.bfloat16

_compiled = {}


def _build_nc():
    nc = bacc.Bacc("TRN2")
    x_in = nc.dram_tensor("x", [ROWS, N_STATE], F32R, kind="ExternalInput")
    # one combined constant block: [identity(128) | RR(256) | ccss(384)]
    # where RR holds per-head [R_even+R_odd | R_even | R_odd] blocks (zero
    # padded to 256 so the f32r matmul keeps a 256-wide moving dim) and
    # ccss[p, c] = [cos32 | sin-cos | cos+sin] per position c*128+p
    consts_in = nc.dram_tensor("consts", [128, 768], F32R,
                               kind="ExternalInput")
    out_d = nc.dram_tensor("out", [ROWS, N_STATE], F32, kind="ExternalOutput")

    with tile.TileContext(nc) as tc:
        with (
            tc.tile_pool(name="const", bufs=1) as const,
            tc.tile_pool(name="xin", bufs=2 * NBLK) as xin,
            tc.tile_pool(name="xtp", bufs=3) as xtp,
            tc.tile_pool(name="tpsum", bufs=2, space="PSUM") as tpsum,
            tc.tile_pool(name="ypsum", bufs=2, space="PSUM") as ypsum,
            tc.tile_pool(name="tsb", bufs=3) as tsb,
            # one buffer per subtile: output buffers must never gate compute
            # (stores drain late because loads occupy the DMA engines first)
            tc.tile_pool(name="outp", bufs=2 * NBLK) as outp,
        ):
            # first subtile load goes ahead of everything else, split in two
            # so the transpose->copy->matmul->mul chain starts as early as
            # possible (chunks 0-3 usable after a 256 KB transfer)
            xts = []
            x_t = xin.tile([128, N_STATE], F32R, tag="xin", name="xs0")
            nc.sync.dma_start(out=x_t[:, 0:512], in_=x_in[0:128, 0:512])
            xts.append(x_t)

            consts = const.tile([128, 768], F32R)
            nc.sync.dma_start(out=consts[:], in_=consts_in[:])
            nc.sync.dma_start(out=x_t[:, 512:1024], in_=x_in[0:128, 512:1024])
            cv = consts[:]

            # remaining subtile loads, all upfront: store waits can never
            # delay a load (single SP queue, loads issued first); subtiles
            # 1-2 split in half so the still-filling pipeline starves less
            for st in range(1, 2 * NBLK):
                xts.append(xin.tile([128, N_STATE], F32R, tag="xin",
                                    name=f"xs{st}"))
            # subtiles 1-2 split in half so the filling pipeline
            # starves less
            for st in (1, 2):
                nc.sync.dma_start(out=xts[st][:, 0:512],
                                  in_=x_in[st * 128:(st + 1) * 128, 0:512])
                nc.sync.dma_start(out=xts[st][:, 512:1024],
                                  in_=x_in[st * 128:(st + 1) * 128, 512:1024])
            for st in range(3, 2 * NBLK):
                nc.sync.dma_start(out=xts[st][:],
                                  in_=x_in[st * 128:(st + 1) * 128, :])

            for blk in range(NBLK):
                for j in range(2):
                    x_t = xts[2 * blk + j]
                    c = (2 * blk + j) % CBLK
                    xT = xtp.tile([128, NCH, 128], F32R)
                    t_sb = tsb.tile([128, NCH, 192], F32)
                    tb = t_sb[:]
                    o_t = outp.tile([128, N_STATE], F32)
                    ob = o_t[:]
                    def transpose_group(g):
                        # transpose 4 chunks: feats -> partitions
                        tp = tpsum.tile([128, 4, 128], F32R, tag="tp")
                        for q in range(4):
                            ch = 4 * g + q
                            nc.tensor.transpose(
                                tp[:, q, :],
                                x_t[:, ch * 128:(ch + 1) * 128],
                                consts[:, 0:128],
                            )
                        nc.scalar.copy(out=xT[:, 4 * g:4 * (g + 1), :],
                                       in_=tp[:])

                    def transpose_chunks(g0, gn):
                        # per-mul-group staging: shortest startup chain
                        tp = tpsum.tile([128, 4, 128], F32R, tag="tp")
                        for a in range(gn):
                            ch = g0 + a
                            nc.tensor.transpose(
                                tp[:, a, :],
                                x_t[:, ch * 128:(ch + 1) * 128],
                                consts[:, 0:128],
                            )
                        nc.scalar.copy(out=xT[:, g0:g0 + gn, :],
                                       in_=tp[:, 0:gn, :])

                    first = (blk == 0 and j == 0)
                    if not first:
                        transpose_group(0)
                    adds_done = 0
                    groups = ((0, 2), (2, 2), (4, 2), (6, 2)) if first else \
                        ((0, 3), (3, 3), (6, 2))
                    for g0, gn in groups:
                        if first:
                            transpose_chunks(g0, gn)
                        elif g0 == 3:
                            # second transpose group after the first matmul
                            # group: PE unblocks DVE's group-0 mul earliest
                            transpose_group(1)
                        # [P|Q] per chunk; one matmul per PSUM bank
                        pq = ypsum.tile([128, 3, 512], F32)
                        for a in range(gn):
                            nc.tensor.matmul(
                                pq[:, a, 0:256], xT[:, g0 + a, :], consts[:, 128:384],
                                start=True, stop=True,
                            )
                        pqb = pq[:]
                        # k = [z1*c | z2*(s-c) | z3*(c+s)] per head (96 each)
                        nc.vector.tensor_mul(
                            bass.AP(tensor=tb.tensor,
                                    offset=tb.offset + g0 * 192,
                                    ap=[list(tb.ap[0]), [192, gn], [96, 2],
                                        [1, 96]]),
                            bass.AP(tensor=pqb.tensor, offset=pqb.offset,
                                    ap=[list(pqb.ap[0]), [512, gn], [96, 2],
                                        [1, 96]]),
                            bass.AP(tensor=cv.tensor,
                                    offset=cv.offset + 384 + c * 96,
                                    ap=[list(cv.ap[0]), [0, gn], [0, 2],
                                        [1, 96]]).bitcast(F32),
                        )
                        # ready chunks -> add + store; half-subtile granules
                        # normally, quarters on the final subtile so the tail
                        # after the very last mul is as short as possible
                        final = (blk == NBLK - 1 and j == 1)
                        gran = 4
                        while (g0 + gn) >= gran * (adds_done + 1):
                            h = adds_done
                            w = gran * 128
                            tb0 = tb.offset + h * gran * 192
                            # out_lo = k1 - k3 ; out_hi = k1 + k2; one of the
                            # four add-ops per subtile runs on DVE (Pool would
                            # otherwise pace the run), and the final subtile's
                            # trailing pair skips Pool's backlog entirely
                            for oi, (dcol, din, op) in enumerate(
                                    ((0, 64, mybir.AluOpType.subtract),
                                     (32, 32, mybir.AluOpType.add))):
                                dve = (final and h == 1) or \
                                    (h == 0 and oi == 0)
                                eng = nc.vector if dve else nc.gpsimd
                                eng.tensor_tensor(
                                    out=bass.AP(tensor=ob.tensor,
                                                offset=ob.offset + h * w + dcol,
                                                ap=[list(ob.ap[0]),
                                                    [128, gran], [64, 2],
                                                    [1, HALF]]),
                                    in0=bass.AP(tensor=tb.tensor, offset=tb0,
                                                ap=[list(tb.ap[0]),
                                                    [192, gran], [96, 2],
                                                    [1, HALF]]),
                                    in1=bass.AP(tensor=tb.tensor,
                                                offset=tb0 + din,
                                                ap=[list(tb.ap[0]),
                                                    [192, gran], [96, 2],
                                                    [1, HALF]]),
                                    op=op,
                                )
                            r0 = blk * 256 + j * 128
                            if h == 1:
                                # DMA-bound regime: one full-subtile store
                                # packs the (now binding) DMA stream best
                                nc.sync.dma_start(out=out_d[r0:r0 + 128, :],
                                                  in_=o_t[:])
                            adds_done += 1
    nc.compile()  # bacc: splits multi-sem waits into EventSemaphore insts
    return nc


def _compose_r(thetas, rotation_pairs, theta_scale, rotation_matrix):
    """Replicates reference._compose_rotation."""
    idx = rotation_pairs.astype(np.int32)
    th = thetas.astype(np.float32) * np.float32(theta_scale[0])
    R = np.eye(D, dtype=np.float32)
    for k in range(th.shape[0]):
        i, j = int(idx[k, 0]), int(idx[k, 1])
        ck, sk = np.float32(np.cos(th[k])), np.float32(np.sin(th[k]))
        G = np.eye(D, dtype=np.float32)
        G[i, i] = ck
        G[i, j] = -sk
        G[j, i] = sk
        G[j, j] = ck
        R = (R @ G).astype(np.float32)
    return (R @ rotation_matrix.astype(np.float32)).astype(np.float32)


def _build_rr(R):
    """[128, 256]: per head h (rows 64h:64h+64, cols 96h:96h+96) the block
    [R_even+R_odd | R_even | R_odd] -> z1 = u+v, z2 = u, z3 = v. Columns
    192:256 are zero padding so the f32r matmul keeps its 256-wide (1
    cycle/row) moving dimension."""
    w = np.concatenate([R[:, 0::2] + R[:, 1::2], R[:, 0::2], R[:, 1::2]],
                       axis=1).astype(np.float32)
    rr = np.zeros((128, 256), dtype=np.float32)
    rr[0:D, 0:96] = w
    rr[D:128, 96:192] = w
    return np.ascontiguousarray(rr)


def make_in_maps(x, thetas, rotation_pairs, theta_scale, rotation_matrix,
                 inv_freq):
    x = np.asarray(x, dtype=np.float32)
    R = _compose_r(
        np.asarray(thetas, np.float32),
        np.asarray(rotation_pairs, np.float32),
        np.asarray(theta_scale, np.float32),
        np.asarray(rotation_matrix, np.float32),
    )
    rr = _build_rr(R)
    invf = np.asarray(inv_freq, np.float32)
    pos = np.arange(S, dtype=np.float32)
    sinusoid = pos[:, None] * invf[None, :]               # [S, 32]
    cosf = np.cos(sinusoid).astype(np.float32)
    sinf = np.sin(sinusoid).astype(np.float32)

    in_maps = []
    for k in range(N_CORES):
        blk = slice(k * S_SH, (k + 1) * S_SH)
        # ccss[p, c] = [cos | sin-cos | cos+sin] (pos = c*128 + p); with
        # k1 = (u+v)cos, k2 = u(sin-cos), k3 = v(cos+sin):
        # out_lo = k1-k3 = u cos - v sin, out_hi = k1+k2 = u sin + v cos
        cc = cosf[blk].reshape(CBLK, 128, HALF)
        ss = sinf[blk].reshape(CBLK, 128, HALF)
        ccss = np.ascontiguousarray(
            np.concatenate([cc, ss - cc, cc + ss], axis=2).transpose(1, 0, 2),
            dtype=np.float32)
        xs = np.ascontiguousarray(x[:, blk, :]).reshape(ROWS, N_STATE)
        consts = np.concatenate(
            [np.eye(128, dtype=np.float32), rr, ccss.reshape(128, CBLK * 96)],
            axis=1)
        in_maps.append({"x": xs, "consts": np.ascontiguousarray(consts)})
    return in_maps


def kernel(x, thetas, rotation_pairs, theta_scale, rotation_matrix, inv_freq):
    in_maps = make_in_maps(x, thetas, rotation_pairs, theta_scale,
                           rotation_matrix, inv_freq)
    if "nc" not in _compiled:
        _compiled["nc"] = _build_nc()
    res = run_bass_kernel_spmd(_compiled["nc"], in_maps,
                               list(range(N_CORES))).results

    out = np.empty((B, S, N_STATE), dtype=np.float32)
    for k in range(N_CORES):
        blk = slice(k * S_SH, (k + 1) * S_SH)
        out[:, blk, :] = res[k]["out"].reshape(B, S_SH, N_STATE)
    return out
